# revision 24
# baseline (speedup 1.0000x reference)
"""Trainium2 Bass kernel for nn_EuclideanDistance (retrieval_knn).

reference: out = quantize(x_pad) @ quantize(temp)
  where temp  = [weight; broadcast(bias, L rows)],  bias = colsum(weight^2)/L
        x_pad = [x, ones(B, L)]
        quantize(t) = round(t/s)*s,  s = max(max|t|/127, 1e-12)  (per tensor)

Strategy: shard the stored-vector axis N=16384 across 8 cores (2048 each),
replicate x. The correctness gate is rel_err < 2e-2 Frobenius; the
reference's own 8-bit quantization noise is ~2e-3 of the output, so the
device matmul runs in fp8 (e4m3) DoubleRow mode at ~2x the bf16 PE rate:

  device:  P = e4m3(x) @ e4m3(W)           (fp8 in, fp8 out, P^T layout)
  host:    out[b,n] = f32(P8[n,b]) + c[n]
  c[n] = L*round(1/sx)*round(bias[n]/sw)*sx*sw   (exact replication of the
         reference's ones x bias-rows term, constant across the batch)

Divergence from the reference is fp8-vs-int8 rounding noise in x@W plus the
fp8 output store: measured 2.9e-3 rel Frobenius on the real input
distribution (7x inside the gate). |P| <= ~120 < 240 so e4m3 never clips.

All quantization and data layout happens on HOST (ml_dtypes.float8_e4m3
bit-matches TRN FP8_EXP4 for |v|<=240). Device-side schedule (all times
relative to the measured exec window):

* Loads ride the qSPDynamicHW HWDGE ring, which drains queued transfers
  in FIFO issue order (trace-verified), so no drain is needed: the gate
  (w nb0 + x b0, fused into ONE 512KB 4KB-line DMA via the kq slot
  layout [nb1,nb2,nb3,nb0,xb0,xb1]) completes first, then x b1 (phase
  A2), w nb1 (j4), w nb2-3. Store issues queue behind the loads on the
  same ring and cannot dilute them.
* PE clock ramp: the HAM flips K=4/8 (1.2GHz) -> 8/8 (2.4GHz) only
  after ~3.4-4.2us of sustained PE-array busy in a free-running 4096-
  cycle window; an idle gap >~0.3us before the flip postpones it into
  the real phase (measured +0.9us for a 0.3us gap, +2us for a 2us gap).
  The warmup therefore (a) reads a RAW pre-tile SBUF tensor with no
  producer (garbage bits; PE timing is data-independent) so it starts
  right at PE tile-entry with zero cross-engine jitter, and (b) tapers
  with 64-col matmuls sized to bridge to the gate landing on the
  slowest core. Real matmuls then run at the fp8 DoubleRow hardware
  peak: 215ns per [128x512, K=256x2] chunk-half = 156 TF/s, 64 matmuls
  = 13.8us, back-to-back.
* psum evacuation (pure f32->fp8 cast) alternates DVE (b0 halves) and
  ACT (b1 halves); ob staging is 8-deep so evacs never wait on store
  completions. Stores are paired 256KB DMAs; the last two chunks store
  per-chunk, and the very last chunk splits across BOTH HWDGE rings
  (b0 half issued from ACT/qActDynamicHW, b1 half from SP) so the two
  issue+DGE chains run in parallel.
* A fixed ~11.3us closes every run: DMA receipt, tile-end barriers,
  and a walrus-injected epilogue that zeroes the entire 253-semaphore
  file one instruction per sem per engine (~6.3us, PE slowest at
  ~115ns/sem) -- NEFF-level, not controllable from the kernel.
"""

import sys
import time

import numpy as np

try:
    import concourse.bacc as bacc  # noqa: F401
except ImportError:  # fresh interpreter without the repo on sys.path
    sys.path.insert(0, "/opt/trn_rl_repo")

import ml_dtypes

import concourse.bacc as bacc
import concourse.mybir as mybir
import concourse.tile as tile
from concourse import bass_utils

B, D, N = 1024, 512, 16384
NCORES = 8
NS = N // NCORES          # 2048 stored vectors per core
L = 32                    # split_square_len
QMAX = np.float32(127.0)  # 2**(8-1) - 1
KC = D // 128             # 4 K-chunks (2 DoubleRow pairs)
NC = NS // 128            # 16 output-partition chunks
NB = NS // 512            # 4 n-blocks (one per 512 weight columns)
BT = B // 512             # 2 rhs tiles
NWARM_BIG = 15            # 256-col PE clock-ramp matmuls (~3.2us at 1.2GHz)
NWARM_SMALL = 24          # 64-col taper: bridges warmup end to the gate
                          # landing (~gm+5.7) -- an idle PE gap >~0.3us
                          # degrades the HAM busy window and postpones the
                          # 2.4GHz flip into the real phase (measured +2us
                          # for a 2us gap, +0.9 for a 0.3us gap)
WPOS = (3, 0, 1, 2)       # logical w block -> physical slot in kq (nb0 last,
                          # adjacent to x b0, so the gate is ONE 4KB-line DMA)

F32 = mybir.dt.float32
BF16 = mybir.dt.bfloat16
FP8 = mybir.dt.float8e4

E4M3 = ml_dtypes.float8_e4m3

_NC_CACHE = None


def _body(nc, tc, g8, xb1_8, wt8, outT, wrm):
    from contextlib import ExitStack

    ID = mybir.ActivationFunctionType.Identity
    DR = mybir.MatmulPerfMode.DoubleRow

    with ExitStack() as ctx:
        qpool = ctx.enter_context(tc.tile_pool(name="qk", bufs=1))
        ppool = ctx.enter_context(tc.tile_pool(name="psum", bufs=8, space="PSUM"))
        opool = ctx.enter_context(tc.tile_pool(name="osb", bufs=8))

        # one fused operand tile: slots 0-2 = w nb1-3, slot 3 = w nb0,
        # slot 4 = x b0, slot 5 = x b1.  nb0|xb0 adjacency makes the
        # first-matmul gate a single contiguous 512KB DMA with 4KB
        # per-partition lines (~0.5us faster than two 2KB-line DMAs).
        kq = qpool.tile([128, 6, KC, 512], FP8, name="kq")

        # ---- loads: the HWDGE ring (qSPDynamicHW) drains queued transfers
        #      in FIFO issue order (trace-verified: load sems fire strictly
        #      sequentially), so the gate completes first without a drain,
        #      then x b1 (phase A2), w nb1 (j4), w nb2-3 (j8/j12) in
        #      deadline order. Store issues queue behind and cannot dilute
        #      the loads. ----
        nc.sync.dma_start(kq[:, 3:5], g8)
        nc.sync.dma_start(kq[:, 5:6], xb1_8)
        nc.sync.dma_start(kq[:, 0:1], wt8[:, 0:1])
        nc.sync.dma_start(kq[:, 1:3], wt8[:, 1:3])

        # ---- PE warm-up: dummy matmuls ramp the PE clock (HAM flips K=4->8
        #      after ~3.4-4.1us of sustained PE busy, free-running window).
        #      wrm is a RAW pre-tile SBUF tensor with NO producer: garbage
        #      bits are fine (PE timing is data-independent, results
        #      discarded), so the warmup starts right at PE tile-entry with
        #      zero cross-engine dependency (tile-pool memsets measured
        #      jittering the warmup start by up to 1.2us across cores). ----
        ps_warm = ppool.tile([128, 512], F32, name="ps", tag="ps", bufs=8)
        for _ in range(NWARM_BIG):
            nc.tensor.matmul(ps_warm[:, 0:256], wrm[:, 0:128],
                             wrm[:, 128:384], start=True, stop=True)
        for _ in range(NWARM_SMALL):
            nc.tensor.matmul(ps_warm[:, 0:64], wrm[:, 0:128],
                             wrm[:, 128:192], start=True, stop=True)

        def mm(ps, j, b, i):
            nc.tensor.matmul(
                ps,
                kq[:, WPOS[j // 4], 2 * i:2 * i + 2,
                   (j % 4) * 128:(j % 4) * 128 + 128],
                kq[:, 4 + b, 2 * i:2 * i + 2, :],
                start=(i == 0), stop=(i == 1), perf_mode=DR)

        def psh():
            return ppool.tile([128, 512], F32, name="ps", tag="ps", bufs=8)

        def evac(obs, ps, on_dve):
            if on_dve:
                nc.vector.tensor_copy(obs, ps)
            else:
                nc.scalar.activation(obs, ps, ID)

        # ---- phase A: j0-3 on the b0 half only (x b1 still in flight),
        #      interleaved across j so accumulate chains don't stall.
        #      psum is 8 single-bank [128,512] tiles, one per (j,b) group:
        #      twice the WAR slack of paired banks, and every half
        #      evacuates right after its 2nd matmul ----
        psA = [psh() for _ in range(4)]
        for i in range(2):
            for j in range(4):
                mm(psA[j], j, 0, i)

        # ---- phase A2/B: finish j0-3 on b1, then j4-15 in the order
        #      b0i0, b1i0, b0i1, b1i1 (accumulate chains separated by one
        #      matmul; the b0 half closes on the 3rd matmul and evacuates
        #      while b1 finishes). Half-evacs go b0->DVE, b1->ACT; stores
        #      are paired 256KB DMAs, the last two chunks store per-half
        #      the moment each evac lands: the post-matmul drain tail is
        #      one half-evac + one issue + 64KB ----
        for jp in range(NC // 2):
            last = jp == NC // 2 - 1
            ob = opool.tile([128, 2 * B], FP8, name="ob", tag="ob", bufs=8)
            for h in range(2):
                j = jp * 2 + h
                obs = ob[:, h * B:(h + 1) * B]
                if j < 4:
                    evac(obs[:, 0:512], psA[j], on_dve=True)
                    ps1 = psh()
                    for i in range(2):
                        mm(ps1, j, 1, i)
                    evac(obs[:, 512:B], ps1, on_dve=False)
                else:
                    ps0, ps1 = psh(), psh()
                    mm(ps0, j, 0, 0)
                    mm(ps1, j, 1, 0)
                    mm(ps0, j, 0, 1)
                    final = last and h == 1
                    if not final:
                        evac(obs[:, 0:512], ps0, on_dve=True)
                        mm(ps1, j, 1, 1)
                        evac(obs[:, 512:B], ps1, on_dve=False)
                        if last:
                            # j14: one 128KB store; the sync issue chain
                            # (~0.63us), not the evacs, bounds this chunk
                            nc.sync.dma_start(outT[:, j, :], obs)
                    else:
                        # very last chunk: b0 half evacs on ACT and stores
                        # via the ACT HWDGE ring (qActDynamicHW) while the
                        # b1 half evacs on the otherwise-idle DVE and
                        # stores 64KB on the SP ring -- two parallel issue
                        # chains cut ~0.4us off the drain tail
                        evac(obs[:, 0:512], ps0, on_dve=False)
                        mm(ps1, j, 1, 1)
                        nc.scalar.dma_start(outT[:, j, 0:512],
                                            obs[:, 0:512])
                        evac(obs[:, 512:B], ps1, on_dve=True)
                        nc.sync.dma_start(outT[:, j, 512:B],
                                          obs[:, 512:B])
            if not last:
                nc.sync.dma_start(
                    outT[:, jp * 2:(jp + 1) * 2, :],
                    ob.rearrange("p (a c) -> p a c", a=2))


def _build():
    global _NC_CACHE
    if _NC_CACHE is not None:
        return _NC_CACHE
    nc = bacc.Bacc("TRN2", target_bir_lowering=False, debug=False,
                   enable_asserts=False, num_devices=1)
    g8 = nc.dram_tensor("g8", [128, 2, KC, 512], FP8,
                        kind="ExternalInput").ap()
    xb1_8 = nc.dram_tensor("xb1", [128, 1, KC, 512], FP8,
                           kind="ExternalInput").ap()
    wt8 = nc.dram_tensor("wt8", [128, 3, KC, 512], FP8,
                         kind="ExternalInput").ap()
    outT = nc.dram_tensor("outT", [128, NC, B], FP8,
                          kind="ExternalOutput").ap()
    # raw (non-tile) warmup operand: read-only garbage, no producer
    wrm = nc.alloc_sbuf_tensor("wrm0", [128, 384], BF16).ap()
    with tile.TileContext(nc) as tc:
        _body(nc, tc, g8, xb1_8, wt8, outT, wrm)
    nc.compile()
    _NC_CACHE = nc
    return nc


def _prepare_inputs(x, weight, split_square_len):
    assert x.shape == (B, D) and weight.shape == (D, N)
    assert int(split_square_len) == L

    x = np.ascontiguousarray(x, dtype=np.float32)
    weight = np.ascontiguousarray(weight, dtype=np.float32)

    # bias = colsum(weight^2)/L in f32, matching the reference
    bias = (np.einsum("dn,dn->n", weight, weight, dtype=np.float32)
            / np.float32(L)).astype(np.float32)

    # reference's global per-tensor scales (f32 arithmetic to match jax)
    max_x = np.float32(max(np.abs(x).max(), np.float32(1.0)))
    sx = np.maximum(max_x / QMAX, np.float32(1e-12))
    max_w = np.float32(max(np.abs(weight).max(), np.abs(bias).max()))
    sw = np.maximum(max_w / QMAX, np.float32(1e-12))

    # ones/bias rank-1 term: c[n] = L * round(1/sx) * round(bias[n]/sw)
    # * sx*sw --- exact replication of the reference's bias-rows term,
    # added on HOST after the fp8 store (values ~512 would swamp e4m3).
    k1 = np.float32(np.round(np.float32(1.0) / sx))
    kb = np.round(bias / sw).astype(np.float32)
    c_scaled = (np.float32(L) * k1) * kb * (sx * sw)

    # block-packed SBUF layouts: [p, blk, k, col] with 2KB+ lines
    xT = np.ascontiguousarray(x.T).astype(E4M3)          # [D, B]
    x8p = np.ascontiguousarray(
        xT.reshape(KC, 128, BT, 512).transpose(1, 2, 0, 3))
    w_q = weight.astype(E4M3)                            # [D, N]

    in_maps = []
    for c in range(NCORES):
        wc = w_q[:, c * NS:(c + 1) * NS]                 # [D, NS]
        w8p = wc.reshape(KC, 128, NB, 512).transpose(1, 2, 0, 3)
        # gate = [w nb0 | x b0] fused into one 4KB-line transfer
        g8 = np.ascontiguousarray(
            np.stack([w8p[:, 0], x8p[:, 0]], axis=1))    # [128,2,KC,512]
        xb1 = np.ascontiguousarray(x8p[:, 1:2])          # [128,1,KC,512]
        wt8 = np.ascontiguousarray(w8p[:, 1:4])          # [128,3,KC,512]
        in_maps.append({"g8": g8, "xb1": xb1, "wt8": wt8})
    return in_maps, c_scaled


def _run(in_maps, **kwargs):
    nc = _build()
    return bass_utils.run_bass_kernel_spmd(
        nc, in_maps, core_ids=list(range(NCORES)), **kwargs)


def _finalize(res, c_scaled):
    parts = []
    for c in range(NCORES):
        o = res.results[c]["outT"]                   # [128, NC, B] fp8
        parts.append(np.asarray(o).transpose(1, 0, 2).reshape(NS, B))
    out = np.concatenate(parts, axis=0).astype(np.float32)   # [N, B]
    out += c_scaled[:, None]
    return np.ascontiguousarray(out.T)               # [B, N] f32


def kernel(x, weight, split_square_len):
    in_maps, c_scaled = _prepare_inputs(x, weight, split_square_len)
    res = None
    for attempt in range(3):
        try:
            res = _run(in_maps)
            break
        except Exception:
            # transient NRT_EXEC_UNIT_UNRECOVERABLE device wedges have been
            # observed on this fabric; a plain re-execute does not always
            # clear them, but tearing down the PJRT client (the in-process
            # equivalent of a fresh interpreter) does
            if attempt == 2:
                raise
            try:
                import jax
                import jax.extend as _jex
                jax.clear_caches()
                _jex.backend.clear_backends()
            except Exception:
                pass
            time.sleep(2.0)
    return _finalize(res, c_scaled)



# revision 25
# speedup vs baseline: 1.0099x; 1.0099x over previous
"""Trainium2 Bass kernel for nn_EuclideanDistance (retrieval_knn).

reference: out = quantize(x_pad) @ quantize(temp)
  where temp  = [weight; broadcast(bias, L rows)],  bias = colsum(weight^2)/L
        x_pad = [x, ones(B, L)]
        quantize(t) = round(t/s)*s,  s = max(max|t|/127, 1e-12)  (per tensor)

Strategy: shard the stored-vector axis N=16384 across 8 cores (2048 each),
replicate x. The correctness gate is rel_err < 2e-2 Frobenius; the
reference's own 8-bit quantization noise is ~2e-3 of the output, so the
device matmul runs in fp8 (e4m3) DoubleRow mode at ~2x the bf16 PE rate:

  device:  P = e4m3(x) @ e4m3(W)           (fp8 in, fp8 out, P^T layout)
  host:    out[b,n] = f32(P8[n,b]) + c[n]
  c[n] = L*round(1/sx)*round(bias[n]/sw)*sx*sw   (exact replication of the
         reference's ones x bias-rows term, constant across the batch)

Divergence from the reference is fp8-vs-int8 rounding noise in x@W plus the
fp8 output store: measured 2.9e-3 rel Frobenius on the real input
distribution (7x inside the gate). |P| <= ~120 < 240 so e4m3 never clips.

All quantization and data layout happens on HOST (ml_dtypes.float8_e4m3
bit-matches TRN FP8_EXP4 for |v|<=240). Device-side schedule (all times
relative to the measured exec window):

* Loads ride the qSPDynamicHW HWDGE ring, which drains queued transfers
  in FIFO issue order (trace-verified), so no drain is needed: the gate
  (w nb0 + x b0, fused into ONE 512KB 4KB-line DMA via the kq slot
  layout [nb1,nb2,nb3,nb0,xb0,xb1]) completes first, then x b1 (phase
  A2), w nb1 (j4), w nb2-3. Store issues queue behind the loads on the
  same ring and cannot dilute them.
* PE clock ramp: the HAM flips K=4/8 (1.2GHz) -> 8/8 (2.4GHz) only
  after ~3.4-4.2us of sustained PE-array busy in a free-running 4096-
  cycle window; an idle gap >~0.3us before the flip postpones it into
  the real phase (measured +0.9us for a 0.3us gap, +2us for a 2us gap).
  The warmup therefore (a) reads a RAW pre-tile SBUF tensor with no
  producer (garbage bits; PE timing is data-independent) so it starts
  right at PE tile-entry with zero cross-engine jitter, and (b) tapers
  with 64-col matmuls sized to bridge to the gate landing on the
  slowest core. Real matmuls then run at the fp8 DoubleRow hardware
  peak: 215ns per [128x512, K=256x2] chunk-half = 156 TF/s, 64 matmuls
  = 13.8us, back-to-back.
* psum evacuation (pure f32->fp8 cast) alternates DVE (b0 halves) and
  ACT (b1 halves); ob staging is 8-deep so evacs never wait on store
  completions. Stores are paired 256KB DMAs; the last two chunks store
  per-chunk, and the very last chunk splits across BOTH HWDGE rings
  (b0 half issued from ACT/qActDynamicHW, b1 half from SP) so the two
  issue+DGE chains run in parallel.
* A fixed ~11.3us closes every run: DMA receipt, tile-end barriers,
  and a walrus-injected epilogue that zeroes the entire 253-semaphore
  file one instruction per sem per engine (~6.3us, PE slowest at
  ~115ns/sem) -- NEFF-level, not controllable from the kernel.
"""

import sys
import time

import numpy as np

try:
    import concourse.bacc as bacc  # noqa: F401
except ImportError:  # fresh interpreter without the repo on sys.path
    sys.path.insert(0, "/opt/trn_rl_repo")

import ml_dtypes

import concourse.bacc as bacc
import concourse.mybir as mybir
import concourse.tile as tile
from concourse import bass_utils

B, D, N = 1024, 512, 16384
NCORES = 8
NS = N // NCORES          # 2048 stored vectors per core
L = 32                    # split_square_len
QMAX = np.float32(127.0)  # 2**(8-1) - 1
KC = D // 128             # 4 K-chunks (2 DoubleRow pairs)
NC = NS // 128            # 16 output-partition chunks
NB = NS // 512            # 4 n-blocks (one per 512 weight columns)
BT = B // 512             # 2 rhs tiles
NWARM_BIG = 15            # 256-col PE clock-ramp matmuls (~3.2us at 1.2GHz)
NWARM_SMALL = 30          # 64-col taper: bridges warmup end to the gate
                          # landing (~gm+5.7) -- an idle PE gap >~0.3us
                          # degrades the HAM busy window and postpones the
                          # 2.4GHz flip into the real phase (measured +2us
                          # for a 2us gap, +0.9 for a 0.3us gap)
WPOS = (3, 0, 1, 2)       # logical w block -> physical slot in kq (nb0 last,
                          # adjacent to x b0, so the gate is ONE 4KB-line DMA)

F32 = mybir.dt.float32
BF16 = mybir.dt.bfloat16
FP8 = mybir.dt.float8e4

E4M3 = ml_dtypes.float8_e4m3

_NC_CACHE = None


def _body(nc, tc, g8, xb1_8, wt8, outT, wrm):
    from contextlib import ExitStack

    ID = mybir.ActivationFunctionType.Identity
    DR = mybir.MatmulPerfMode.DoubleRow

    with ExitStack() as ctx:
        qpool = ctx.enter_context(tc.tile_pool(name="qk", bufs=1))
        ppool = ctx.enter_context(tc.tile_pool(name="psum", bufs=8, space="PSUM"))
        opool = ctx.enter_context(tc.tile_pool(name="osb", bufs=8))

        # one fused operand tile: slots 0-2 = w nb1-3, slot 3 = w nb0,
        # slot 4 = x b0, slot 5 = x b1.  nb0|xb0 adjacency makes the
        # first-matmul gate a single contiguous 512KB DMA with 4KB
        # per-partition lines (~0.5us faster than two 2KB-line DMAs).
        kq = qpool.tile([128, 6, KC, 512], FP8, name="kq")

        # ---- loads: the HWDGE ring (qSPDynamicHW) drains queued transfers
        #      in FIFO issue order (trace-verified: load sems fire strictly
        #      sequentially), so the gate completes first without a drain,
        #      then x b1 (phase A2), w nb1 (j4), w nb2-3 (j8/j12) in
        #      deadline order. Store issues queue behind and cannot dilute
        #      the loads. ----
        nc.sync.dma_start(kq[:, 3:5], g8)
        nc.sync.dma_start(kq[:, 5:6], xb1_8)
        nc.sync.dma_start(kq[:, 0:1], wt8[:, 0:1])
        nc.sync.dma_start(kq[:, 1:3], wt8[:, 1:3])

        # ---- PE warm-up: dummy matmuls ramp the PE clock (HAM flips K=4->8
        #      after ~3.4-4.1us of sustained PE busy, free-running window).
        #      wrm is a RAW pre-tile SBUF tensor with NO producer: garbage
        #      bits are fine (PE timing is data-independent, results
        #      discarded), so the warmup starts right at PE tile-entry with
        #      zero cross-engine dependency (tile-pool memsets measured
        #      jittering the warmup start by up to 1.2us across cores). ----
        ps_warm = ppool.tile([128, 512], F32, name="ps", tag="ps", bufs=8)
        for _ in range(NWARM_BIG):
            nc.tensor.matmul(ps_warm[:, 0:256], wrm[:, 0:128],
                             wrm[:, 128:384], start=True, stop=True)
        for _ in range(NWARM_SMALL):
            nc.tensor.matmul(ps_warm[:, 0:64], wrm[:, 0:128],
                             wrm[:, 128:192], start=True, stop=True)

        def mm(ps, j, b, i):
            nc.tensor.matmul(
                ps,
                kq[:, WPOS[j // 4], 2 * i:2 * i + 2,
                   (j % 4) * 128:(j % 4) * 128 + 128],
                kq[:, 4 + b, 2 * i:2 * i + 2, :],
                start=(i == 0), stop=(i == 1), perf_mode=DR)

        def psh():
            return ppool.tile([128, 512], F32, name="ps", tag="ps", bufs=8)

        def evac(obs, ps, on_dve):
            if on_dve:
                nc.vector.tensor_copy(obs, ps)
            else:
                nc.scalar.activation(obs, ps, ID)

        # ---- phase A: j0-3 on the b0 half only (x b1 still in flight),
        #      interleaved across j so accumulate chains don't stall.
        #      psum is 8 single-bank [128,512] tiles, one per (j,b) group:
        #      twice the WAR slack of paired banks, and every half
        #      evacuates right after its 2nd matmul ----
        psA = [psh() for _ in range(4)]
        for i in range(2):
            for j in range(4):
                mm(psA[j], j, 0, i)

        # ---- phase A2/B: finish j0-3 on b1, then j4-15 in the order
        #      b0i0, b1i0, b0i1, b1i1 (accumulate chains separated by one
        #      matmul; the b0 half closes on the 3rd matmul and evacuates
        #      while b1 finishes). Half-evacs go b0->DVE, b1->ACT; stores
        #      are paired 256KB DMAs, the last two chunks store per-half
        #      the moment each evac lands: the post-matmul drain tail is
        #      one half-evac + one issue + 64KB ----
        for jp in range(NC // 2):
            last = jp == NC // 2 - 1
            ob = opool.tile([128, 2 * B], FP8, name="ob", tag="ob", bufs=8)
            for h in range(2):
                j = jp * 2 + h
                obs = ob[:, h * B:(h + 1) * B]
                if j < 4:
                    evac(obs[:, 0:512], psA[j], on_dve=True)
                    ps1 = psh()
                    for i in range(2):
                        mm(ps1, j, 1, i)
                    evac(obs[:, 512:B], ps1, on_dve=False)
                else:
                    ps0, ps1 = psh(), psh()
                    mm(ps0, j, 0, 0)
                    mm(ps1, j, 1, 0)
                    mm(ps0, j, 0, 1)
                    final = last and h == 1
                    if not final:
                        evac(obs[:, 0:512], ps0, on_dve=True)
                        mm(ps1, j, 1, 1)
                        evac(obs[:, 512:B], ps1, on_dve=False)
                        if last:
                            # j14: one 128KB store; the sync issue chain
                            # (~0.63us), not the evacs, bounds this chunk
                            nc.sync.dma_start(outT[:, j, :], obs)
                    else:
                        # very last chunk: b0 half evacs on ACT and stores
                        # via the ACT HWDGE ring (qActDynamicHW) while the
                        # b1 half evacs on the otherwise-idle DVE and
                        # stores 64KB on the SP ring -- two parallel issue
                        # chains cut ~0.4us off the drain tail
                        evac(obs[:, 0:512], ps0, on_dve=False)
                        mm(ps1, j, 1, 1)
                        nc.scalar.dma_start(outT[:, j, 0:512],
                                            obs[:, 0:512])
                        evac(obs[:, 512:B], ps1, on_dve=True)
                        nc.sync.dma_start(outT[:, j, 512:B],
                                          obs[:, 512:B])
            if not last:
                nc.sync.dma_start(
                    outT[:, jp * 2:(jp + 1) * 2, :],
                    ob.rearrange("p (a c) -> p a c", a=2))


def _build():
    global _NC_CACHE
    if _NC_CACHE is not None:
        return _NC_CACHE
    nc = bacc.Bacc("TRN2", target_bir_lowering=False, debug=False,
                   enable_asserts=False, num_devices=1)
    g8 = nc.dram_tensor("g8", [128, 2, KC, 512], FP8,
                        kind="ExternalInput").ap()
    xb1_8 = nc.dram_tensor("xb1", [128, 1, KC, 512], FP8,
                           kind="ExternalInput").ap()
    wt8 = nc.dram_tensor("wt8", [128, 3, KC, 512], FP8,
                         kind="ExternalInput").ap()
    outT = nc.dram_tensor("outT", [128, NC, B], FP8,
                          kind="ExternalOutput").ap()
    # raw (non-tile) warmup operand: read-only garbage, no producer
    wrm = nc.alloc_sbuf_tensor("wrm0", [128, 384], BF16).ap()
    with tile.TileContext(nc) as tc:
        _body(nc, tc, g8, xb1_8, wt8, outT, wrm)
    nc.compile()
    _NC_CACHE = nc
    return nc


def _prepare_inputs(x, weight, split_square_len):
    assert x.shape == (B, D) and weight.shape == (D, N)
    assert int(split_square_len) == L

    x = np.ascontiguousarray(x, dtype=np.float32)
    weight = np.ascontiguousarray(weight, dtype=np.float32)

    # bias = colsum(weight^2)/L in f32, matching the reference
    bias = (np.einsum("dn,dn->n", weight, weight, dtype=np.float32)
            / np.float32(L)).astype(np.float32)

    # reference's global per-tensor scales (f32 arithmetic to match jax)
    max_x = np.float32(max(np.abs(x).max(), np.float32(1.0)))
    sx = np.maximum(max_x / QMAX, np.float32(1e-12))
    max_w = np.float32(max(np.abs(weight).max(), np.abs(bias).max()))
    sw = np.maximum(max_w / QMAX, np.float32(1e-12))

    # ones/bias rank-1 term: c[n] = L * round(1/sx) * round(bias[n]/sw)
    # * sx*sw --- exact replication of the reference's bias-rows term,
    # added on HOST after the fp8 store (values ~512 would swamp e4m3).
    k1 = np.float32(np.round(np.float32(1.0) / sx))
    kb = np.round(bias / sw).astype(np.float32)
    c_scaled = (np.float32(L) * k1) * kb * (sx * sw)

    # block-packed SBUF layouts: [p, blk, k, col] with 2KB+ lines
    xT = np.ascontiguousarray(x.T).astype(E4M3)          # [D, B]
    x8p = np.ascontiguousarray(
        xT.reshape(KC, 128, BT, 512).transpose(1, 2, 0, 3))
    w_q = weight.astype(E4M3)                            # [D, N]

    in_maps = []
    for c in range(NCORES):
        wc = w_q[:, c * NS:(c + 1) * NS]                 # [D, NS]
        w8p = wc.reshape(KC, 128, NB, 512).transpose(1, 2, 0, 3)
        # gate = [w nb0 | x b0] fused into one 4KB-line transfer
        g8 = np.ascontiguousarray(
            np.stack([w8p[:, 0], x8p[:, 0]], axis=1))    # [128,2,KC,512]
        xb1 = np.ascontiguousarray(x8p[:, 1:2])          # [128,1,KC,512]
        wt8 = np.ascontiguousarray(w8p[:, 1:4])          # [128,3,KC,512]
        in_maps.append({"g8": g8, "xb1": xb1, "wt8": wt8})
    return in_maps, c_scaled


def _run(in_maps, **kwargs):
    nc = _build()
    return bass_utils.run_bass_kernel_spmd(
        nc, in_maps, core_ids=list(range(NCORES)), **kwargs)


def _finalize(res, c_scaled):
    parts = []
    for c in range(NCORES):
        o = res.results[c]["outT"]                   # [128, NC, B] fp8
        parts.append(np.asarray(o).transpose(1, 0, 2).reshape(NS, B))
    out = np.concatenate(parts, axis=0).astype(np.float32)   # [N, B]
    out += c_scaled[:, None]
    return np.ascontiguousarray(out.T)               # [B, N] f32


def kernel(x, weight, split_square_len):
    in_maps, c_scaled = _prepare_inputs(x, weight, split_square_len)
    res = None
    for attempt in range(3):
        try:
            res = _run(in_maps)
            break
        except Exception:
            # transient NRT_EXEC_UNIT_UNRECOVERABLE device wedges have been
            # observed on this fabric; a plain re-execute does not always
            # clear them, but tearing down the PJRT client (the in-process
            # equivalent of a fresh interpreter) does
            if attempt == 2:
                raise
            try:
                import jax
                import jax.extend as _jex
                jax.clear_caches()
                _jex.backend.clear_backends()
            except Exception:
                pass
            time.sleep(2.0)
    return _finalize(res, c_scaled)



# revision 27
# speedup vs baseline: 1.0201x; 1.0101x over previous
"""Trainium2 Bass kernel for nn_EuclideanDistance (retrieval_knn).

reference: out = quantize(x_pad) @ quantize(temp)
  where temp  = [weight; broadcast(bias, L rows)],  bias = colsum(weight^2)/L
        x_pad = [x, ones(B, L)]
        quantize(t) = round(t/s)*s,  s = max(max|t|/127, 1e-12)  (per tensor)

Strategy: shard the stored-vector axis N=16384 across 8 cores (2048 each),
replicate x. The correctness gate is rel_err < 2e-2 Frobenius; the
reference's own 8-bit quantization noise is ~2e-3 of the output, so the
device matmul runs in fp8 (e4m3) DoubleRow mode at ~2x the bf16 PE rate:

  device:  P = e4m3(x) @ e4m3(W)           (fp8 in, fp8 out, P^T layout)
  host:    out[b,n] = f32(P8[n,b]) + c[n]
  c[n] = L*round(1/sx)*round(bias[n]/sw)*sx*sw   (exact replication of the
         reference's ones x bias-rows term, constant across the batch)

Divergence from the reference is fp8-vs-int8 rounding noise in x@W plus the
fp8 output store: measured 2.9e-3 rel Frobenius on the real input
distribution (7x inside the gate). |P| <= ~120 < 240 so e4m3 never clips.

All quantization and data layout happens on HOST (ml_dtypes.float8_e4m3
bit-matches TRN FP8_EXP4 for |v|<=240). Device-side schedule (all times
relative to the measured exec window):

* Loads ride the qSPDynamicHW HWDGE ring, which drains queued transfers
  in FIFO issue order (trace-verified), so no drain is needed: the gate
  (w nb0 + x b0, fused into ONE 512KB 4KB-line DMA via the kq slot
  layout [nb1,nb2,nb3,nb0,xb0,xb1]) completes first, then x b1 (phase
  A2), w nb1 (j4), w nb2-3. Store issues queue behind the loads on the
  same ring and cannot dilute them.
* PE clock ramp: the HAM flips K=4/8 (1.2GHz) -> 8/8 (2.4GHz) only
  after ~3.4-4.2us of sustained PE-array busy in a free-running 4096-
  cycle window; an idle gap >~0.3us before the flip postpones it into
  the real phase (measured +0.9us for a 0.3us gap, +2us for a 2us gap).
  The warmup therefore (a) reads a RAW pre-tile SBUF tensor with no
  producer (garbage bits; PE timing is data-independent) so it starts
  right at PE tile-entry with zero cross-engine jitter, and (b) tapers
  with 64-col matmuls sized to bridge to the gate landing on the
  slowest core. Real matmuls then run at the fp8 DoubleRow hardware
  peak: 215ns per [128x512, K=256x2] chunk-half = 156 TF/s, 64 matmuls
  = 13.8us, back-to-back.
* psum evacuation (pure f32->fp8 cast) alternates DVE (b0 halves) and
  ACT (b1 halves); ob staging is 8-deep so evacs never wait on store
  completions. Stores are paired 256KB DMAs; the last two chunks store
  per-chunk, and the very last chunk splits across BOTH HWDGE rings
  (b0 half issued from ACT/qActDynamicHW, b1 half from SP) so the two
  issue+DGE chains run in parallel.
* A fixed ~11.3us closes every run: DMA receipt, tile-end barriers,
  and a walrus-injected epilogue that zeroes the entire 253-semaphore
  file one instruction per sem per engine (~6.3us, PE slowest at
  ~115ns/sem) -- NEFF-level, not controllable from the kernel.
"""

import sys
import time

import numpy as np

try:
    import concourse.bacc as bacc  # noqa: F401
except ImportError:  # fresh interpreter without the repo on sys.path
    sys.path.insert(0, "/opt/trn_rl_repo")

import ml_dtypes

import concourse.bacc as bacc
import concourse.mybir as mybir
import concourse.tile as tile
from concourse import bass_utils

B, D, N = 1024, 512, 16384
NCORES = 8
NS = N // NCORES          # 2048 stored vectors per core
L = 32                    # split_square_len
QMAX = np.float32(127.0)  # 2**(8-1) - 1
KC = D // 128             # 4 K-chunks (2 DoubleRow pairs)
NC = NS // 128            # 16 output-partition chunks
NB = NS // 512            # 4 n-blocks (one per 512 weight columns)
BT = B // 512             # 2 rhs tiles
NWARM_BIG = 15            # 256-col PE clock-ramp matmuls (~3.2us at 1.2GHz)
NWARM_SMALL = 24          # 64-col taper: bridges warmup end to the gate
                          # landing (~gm+5.7) -- an idle PE gap >~0.3us
                          # degrades the HAM busy window and postpones the
                          # 2.4GHz flip into the real phase (measured +2us
                          # for a 2us gap, +0.9 for a 0.3us gap)
WPOS = (3, 0, 1, 2)       # logical w block -> physical slot in kq (nb0 last,
                          # adjacent to x b0, so the gate is ONE 4KB-line DMA)

F32 = mybir.dt.float32
BF16 = mybir.dt.bfloat16
FP8 = mybir.dt.float8e4

E4M3 = ml_dtypes.float8_e4m3

_NC_CACHE = None


def _body(nc, tc, g8, xb1_8, wt8, outT, wrm):
    from contextlib import ExitStack

    ID = mybir.ActivationFunctionType.Identity
    DR = mybir.MatmulPerfMode.DoubleRow

    with ExitStack() as ctx:
        qpool = ctx.enter_context(tc.tile_pool(name="qk", bufs=1))
        ppool = ctx.enter_context(tc.tile_pool(name="psum", bufs=8, space="PSUM"))
        opool = ctx.enter_context(tc.tile_pool(name="osb", bufs=8))

        # one fused operand tile: slots 0-2 = w nb1-3, slot 3 = w nb0,
        # slot 4 = x b0, slot 5 = x b1.  nb0|xb0 adjacency makes the
        # first-matmul gate a single contiguous 512KB DMA with 4KB
        # per-partition lines (~0.5us faster than two 2KB-line DMAs).
        kq = qpool.tile([128, 6, KC, 512], FP8, name="kq")

        # ---- loads: the HWDGE ring (qSPDynamicHW) drains queued transfers
        #      in FIFO issue order (trace-verified: load sems fire strictly
        #      sequentially), so the gate completes first without a drain,
        #      then x b1 (phase A2), w nb1 (j4), w nb2-3 (j8/j12) in
        #      deadline order. Store issues queue behind and cannot dilute
        #      the loads. ----
        nc.sync.dma_start(kq[:, 3:5], g8)
        nc.sync.dma_start(kq[:, 5:6], xb1_8)
        nc.sync.dma_start(kq[:, 0:1], wt8[:, 0:1])
        nc.sync.dma_start(kq[:, 1:3], wt8[:, 1:3])

        # ---- PE warm-up: dummy matmuls ramp the PE clock (HAM flips K=4->8
        #      after ~3.4-4.1us of sustained PE busy, free-running window).
        #      wrm is a RAW pre-tile SBUF tensor with NO producer: garbage
        #      bits are fine (PE timing is data-independent, results
        #      discarded), so the warmup starts right at PE tile-entry with
        #      zero cross-engine dependency (tile-pool memsets measured
        #      jittering the warmup start by up to 1.2us across cores). ----
        ps_warm = ppool.tile([128, 512], F32, name="ps", tag="ps", bufs=8)
        for _ in range(NWARM_BIG):
            nc.tensor.matmul(ps_warm[:, 0:256], wrm[:, 0:128],
                             wrm[:, 128:384], start=True, stop=True)
        for _ in range(NWARM_SMALL):
            nc.tensor.matmul(ps_warm[:, 0:64], wrm[:, 0:128],
                             wrm[:, 128:192], start=True, stop=True)

        def mm(ps, j, b, i):
            nc.tensor.matmul(
                ps,
                kq[:, WPOS[j // 4], 2 * i:2 * i + 2,
                   (j % 4) * 128:(j % 4) * 128 + 128],
                kq[:, 4 + b, 2 * i:2 * i + 2, :],
                start=(i == 0), stop=(i == 1), perf_mode=DR)

        def psh():
            return ppool.tile([128, 512], F32, name="ps", tag="ps", bufs=8)

        def evac(obs, ps, on_dve):
            if on_dve:
                nc.vector.tensor_copy(obs, ps)
            else:
                nc.scalar.activation(obs, ps, ID)

        # ---- phase A: j0-3 on the b0 half only (x b1 still in flight),
        #      interleaved across j so accumulate chains don't stall.
        #      psum is 8 single-bank [128,512] tiles, one per (j,b) group:
        #      twice the WAR slack of paired banks, and every half
        #      evacuates right after its 2nd matmul ----
        psA = [psh() for _ in range(4)]
        for i in range(2):
            for j in range(4):
                mm(psA[j], j, 0, i)

        # ---- phase A2/B: finish j0-3 on b1, then j4-15 in the order
        #      b0i0, b1i0, b0i1, b1i1 (accumulate chains separated by one
        #      matmul; the b0 half closes on the 3rd matmul and evacuates
        #      while b1 finishes). Half-evacs go b0->DVE, b1->ACT; stores
        #      are paired 256KB DMAs, the last two chunks store per-chunk
        #      with the final chunk split across both HWDGE rings ----
        for jp in range(NC // 2):
            last = jp == NC // 2 - 1
            ob = opool.tile([128, 2 * B], FP8, name="ob", tag="ob", bufs=8)
            for h in range(2):
                j = jp * 2 + h
                obs = ob[:, h * B:(h + 1) * B]
                if j < 4:
                    evac(obs[:, 0:512], psA[j], on_dve=True)
                    ps1 = psh()
                    for i in range(2):
                        mm(ps1, j, 1, i)
                    evac(obs[:, 512:B], ps1, on_dve=False)
                else:
                    ps0, ps1 = psh(), psh()
                    mm(ps0, j, 0, 0)
                    mm(ps1, j, 1, 0)
                    mm(ps0, j, 0, 1)
                    final = last and h == 1
                    if not final:
                        evac(obs[:, 0:512], ps0, on_dve=True)
                        mm(ps1, j, 1, 1)
                        evac(obs[:, 512:B], ps1, on_dve=False)
                        if last:
                            # j14: one 128KB store; the sync issue chain
                            # (~0.63us), not the evacs, bounds this chunk
                            nc.sync.dma_start(outT[:, j, :], obs)
                    else:
                        # very last chunk: b0 half evacs on ACT and stores
                        # via the ACT HWDGE ring (qActDynamicHW) while the
                        # b1 half evacs on the otherwise-idle DVE and
                        # stores 64KB on the SP ring -- two parallel issue
                        # chains cut ~0.4us off the drain tail
                        evac(obs[:, 0:512], ps0, on_dve=False)
                        mm(ps1, j, 1, 1)
                        nc.scalar.dma_start(outT[:, j, 0:512],
                                            obs[:, 0:512])
                        evac(obs[:, 512:B], ps1, on_dve=True)
                        nc.sync.dma_start(outT[:, j, 512:B],
                                          obs[:, 512:B])
            if not last:
                nc.sync.dma_start(
                    outT[:, jp * 2:(jp + 1) * 2, :],
                    ob.rearrange("p (a c) -> p a c", a=2))


def _build():
    global _NC_CACHE
    if _NC_CACHE is not None:
        return _NC_CACHE
    nc = bacc.Bacc("TRN2", target_bir_lowering=False, debug=False,
                   enable_asserts=False, num_devices=1)
    g8 = nc.dram_tensor("g8", [128, 2, KC, 512], FP8,
                        kind="ExternalInput").ap()
    xb1_8 = nc.dram_tensor("xb1", [128, 1, KC, 512], FP8,
                           kind="ExternalInput").ap()
    wt8 = nc.dram_tensor("wt8", [128, 3, KC, 512], FP8,
                         kind="ExternalInput").ap()
    outT = nc.dram_tensor("outT", [128, NC, B], FP8,
                          kind="ExternalOutput").ap()
    # raw (non-tile) warmup operand: read-only garbage, no producer
    wrm = nc.alloc_sbuf_tensor("wrm0", [128, 384], BF16).ap()
    with tile.TileContext(nc) as tc:
        _body(nc, tc, g8, xb1_8, wt8, outT, wrm)
    nc.compile()
    _NC_CACHE = nc
    return nc


def _prepare_inputs(x, weight, split_square_len):
    assert x.shape == (B, D) and weight.shape == (D, N)
    assert int(split_square_len) == L

    x = np.ascontiguousarray(x, dtype=np.float32)
    weight = np.ascontiguousarray(weight, dtype=np.float32)

    # bias = colsum(weight^2)/L in f32, matching the reference
    bias = (np.einsum("dn,dn->n", weight, weight, dtype=np.float32)
            / np.float32(L)).astype(np.float32)

    # reference's global per-tensor scales (f32 arithmetic to match jax)
    max_x = np.float32(max(np.abs(x).max(), np.float32(1.0)))
    sx = np.maximum(max_x / QMAX, np.float32(1e-12))
    max_w = np.float32(max(np.abs(weight).max(), np.abs(bias).max()))
    sw = np.maximum(max_w / QMAX, np.float32(1e-12))

    # ones/bias rank-1 term: c[n] = L * round(1/sx) * round(bias[n]/sw)
    # * sx*sw --- exact replication of the reference's bias-rows term,
    # added on HOST after the fp8 store (values ~512 would swamp e4m3).
    k1 = np.float32(np.round(np.float32(1.0) / sx))
    kb = np.round(bias / sw).astype(np.float32)
    c_scaled = (np.float32(L) * k1) * kb * (sx * sw)

    # block-packed SBUF layouts: [p, blk, k, col] with 2KB+ lines
    xT = np.ascontiguousarray(x.T).astype(E4M3)          # [D, B]
    x8p = np.ascontiguousarray(
        xT.reshape(KC, 128, BT, 512).transpose(1, 2, 0, 3))
    w_q = weight.astype(E4M3)                            # [D, N]

    in_maps = []
    for c in range(NCORES):
        wc = w_q[:, c * NS:(c + 1) * NS]                 # [D, NS]
        w8p = wc.reshape(KC, 128, NB, 512).transpose(1, 2, 0, 3)
        # gate = [w nb0 | x b0] fused into one 4KB-line transfer
        g8 = np.ascontiguousarray(
            np.stack([w8p[:, 0], x8p[:, 0]], axis=1))    # [128,2,KC,512]
        xb1 = np.ascontiguousarray(x8p[:, 1:2])          # [128,1,KC,512]
        wt8 = np.ascontiguousarray(w8p[:, 1:4])          # [128,3,KC,512]
        in_maps.append({"g8": g8, "xb1": xb1, "wt8": wt8})
    return in_maps, c_scaled


def _run(in_maps, **kwargs):
    nc = _build()
    return bass_utils.run_bass_kernel_spmd(
        nc, in_maps, core_ids=list(range(NCORES)), **kwargs)


def _finalize(res, c_scaled):
    parts = []
    for c in range(NCORES):
        o = res.results[c]["outT"]                   # [128, NC, B] fp8
        parts.append(np.asarray(o).transpose(1, 0, 2).reshape(NS, B))
    out = np.concatenate(parts, axis=0).astype(np.float32)   # [N, B]
    out += c_scaled[:, None]
    return np.ascontiguousarray(out.T)               # [B, N] f32


def kernel(x, weight, split_square_len):
    in_maps, c_scaled = _prepare_inputs(x, weight, split_square_len)
    res = None
    for attempt in range(3):
        try:
            res = _run(in_maps)
            break
        except Exception:
            # transient NRT_EXEC_UNIT_UNRECOVERABLE device wedges have been
            # observed on this fabric; a plain re-execute does not always
            # clear them, but tearing down the PJRT client (the in-process
            # equivalent of a fresh interpreter) does
            if attempt == 2:
                raise
            try:
                import jax
                import jax.extend as _jex
                jax.clear_caches()
                _jex.backend.clear_backends()
            except Exception:
                pass
            time.sleep(2.0)
    return _finalize(res, c_scaled)



# revision 29
# speedup vs baseline: 1.0242x; 1.0041x over previous
"""Trainium2 Bass kernel for nn_EuclideanDistance (retrieval_knn).

reference: out = quantize(x_pad) @ quantize(temp)
  where temp  = [weight; broadcast(bias, L rows)],  bias = colsum(weight^2)/L
        x_pad = [x, ones(B, L)]
        quantize(t) = round(t/s)*s,  s = max(max|t|/127, 1e-12)  (per tensor)

Strategy: shard the stored-vector axis N=16384 across 8 cores (2048 each),
replicate x. The correctness gate is rel_err < 2e-2 Frobenius; the
reference's own 8-bit quantization noise is ~2e-3 of the output, so the
device matmul runs in fp8 (e4m3) DoubleRow mode at ~2x the bf16 PE rate:

  device:  P = e4m3(x) @ e4m3(W)           (fp8 in, fp8 out, P^T layout)
  host:    out[b,n] = f32(P8[n,b]) + c[n]
  c[n] = L*round(1/sx)*round(bias[n]/sw)*sx*sw   (exact replication of the
         reference's ones x bias-rows term, constant across the batch)

Divergence from the reference is fp8-vs-int8 rounding noise in x@W plus the
fp8 output store: measured 2.9e-3 rel Frobenius on the real input
distribution (7x inside the gate). |P| <= ~120 < 240 so e4m3 never clips.

All quantization and data layout happens on HOST (ml_dtypes.float8_e4m3
bit-matches TRN FP8_EXP4 for |v|<=240). Device-side schedule (all times
relative to the measured exec window):

* Loads ride the qSPDynamicHW HWDGE ring, which drains queued transfers
  in FIFO issue order (trace-verified), so no drain is needed: the gate
  (w nb0 + x b0, fused into ONE 512KB 4KB-line DMA via the kq slot
  layout [nb1,nb2,nb3,nb0,xb0,xb1]) completes first, then x b1 (phase
  A2), w nb1 (j4), w nb2-3. Store issues queue behind the loads on the
  same ring and cannot dilute them.
* PE clock ramp: the HAM flips K=4/8 (1.2GHz) -> 8/8 (2.4GHz) only
  after ~3.4-4.2us of sustained PE-array busy in a free-running 4096-
  cycle window; an idle gap >~0.3us before the flip postpones it into
  the real phase (measured +0.9us for a 0.3us gap, +2us for a 2us gap).
  The warmup therefore (a) reads a RAW pre-tile SBUF tensor with no
  producer (garbage bits; PE timing is data-independent) so it starts
  right at PE tile-entry with zero cross-engine jitter, and (b) tapers
  with 64-col matmuls sized to bridge to the gate landing on the
  slowest core. Real matmuls then run at the fp8 DoubleRow hardware
  peak: 215ns per [128x512, K=256x2] chunk-half = 156 TF/s, 64 matmuls
  = 13.8us, back-to-back.
* psum evacuation (pure f32->fp8 cast) alternates DVE (b0 halves) and
  ACT (b1 halves); ob staging is 8-deep so evacs never wait on store
  completions. Stores are paired 256KB DMAs; the last two chunks store
  per-chunk, and the very last chunk splits across BOTH HWDGE rings
  (b0 half issued from ACT/qActDynamicHW, b1 half from SP) so the two
  issue+DGE chains run in parallel.
* A fixed ~11.3us closes every run: DMA receipt, tile-end barriers,
  and a walrus-injected epilogue that zeroes the entire 253-semaphore
  file one instruction per sem per engine (~6.3us, PE slowest at
  ~115ns/sem) -- NEFF-level, not controllable from the kernel.
"""

import sys
import time

import numpy as np

try:
    import concourse.bacc as bacc  # noqa: F401
except ImportError:  # fresh interpreter without the repo on sys.path
    sys.path.insert(0, "/opt/trn_rl_repo")

import ml_dtypes

import concourse.bacc as bacc
import concourse.mybir as mybir
import concourse.tile as tile
from concourse import bass_utils

B, D, N = 1024, 512, 16384
NCORES = 8
NS = N // NCORES          # 2048 stored vectors per core
L = 32                    # split_square_len
QMAX = np.float32(127.0)  # 2**(8-1) - 1
KC = D // 128             # 4 K-chunks (2 DoubleRow pairs)
NC = NS // 128            # 16 output-partition chunks
NB = NS // 512            # 4 n-blocks (one per 512 weight columns)
BT = B // 512             # 2 rhs tiles
NWARM_BIG = 15            # 256-col PE clock-ramp matmuls (~3.2us at 1.2GHz)
NWARM_SMALL = 24          # 64-col taper: bridges warmup end to the gate
                          # landing (~gm+5.7) -- an idle PE gap >~0.3us
                          # degrades the HAM busy window and postpones the
                          # 2.4GHz flip into the real phase (measured +2us
                          # for a 2us gap, +0.9 for a 0.3us gap)
WPOS = (3, 0, 1, 2)       # logical w block -> physical slot in kq (nb0 last,
                          # adjacent to x b0, so the gate is ONE 4KB-line DMA)

F32 = mybir.dt.float32
BF16 = mybir.dt.bfloat16
FP8 = mybir.dt.float8e4

E4M3 = ml_dtypes.float8_e4m3

_NC_CACHE = None


def _body(nc, tc, g8, xb1_8, wt8, outT, wrm):
    from contextlib import ExitStack

    ID = mybir.ActivationFunctionType.Identity
    DR = mybir.MatmulPerfMode.DoubleRow

    with ExitStack() as ctx:
        qpool = ctx.enter_context(tc.tile_pool(name="qk", bufs=1))
        ppool = ctx.enter_context(tc.tile_pool(name="psum", bufs=8, space="PSUM"))
        opool = ctx.enter_context(tc.tile_pool(name="osb", bufs=8))

        # one fused operand tile: slots 0-2 = w nb1-3, slot 3 = w nb0,
        # slot 4 = x b0, slot 5 = x b1.  nb0|xb0 adjacency makes the
        # first-matmul gate a single contiguous 512KB DMA with 4KB
        # per-partition lines (~0.5us faster than two 2KB-line DMAs).
        kq = qpool.tile([128, 6, KC, 512], FP8, name="kq")

        # ---- loads: the HWDGE ring (qSPDynamicHW) drains queued transfers
        #      in FIFO issue order (trace-verified: load sems fire strictly
        #      sequentially), so the gate completes first without a drain,
        #      then x b1 (phase A2), w nb1 (j4), w nb2-3 (j8/j12) in
        #      deadline order. Store issues queue behind and cannot dilute
        #      the loads. ----
        # deadline order for the b0-major phase plan: gate (phase A),
        # w nb1 (j4-7 b0 at +1.7us), x b1 (b1 phase at +3.4us -- enough
        # slack to absorb the 1-2us DMA receipt lag), w nb2-3 (j8 at +6.9)
        nc.sync.dma_start(kq[:, 3:5], g8)
        nc.sync.dma_start(kq[:, 0:1], wt8[:, 0:1])
        nc.sync.dma_start(kq[:, 5:6], xb1_8)
        nc.sync.dma_start(kq[:, 1:3], wt8[:, 1:3])

        # ---- PE warm-up: dummy matmuls ramp the PE clock (HAM flips K=4->8
        #      after ~3.4-4.1us of sustained PE busy, free-running window).
        #      wrm is a RAW pre-tile SBUF tensor with NO producer: garbage
        #      bits are fine (PE timing is data-independent, results
        #      discarded), so the warmup starts right at PE tile-entry with
        #      zero cross-engine dependency (tile-pool memsets measured
        #      jittering the warmup start by up to 1.2us across cores). ----
        ps_warm = ppool.tile([128, 512], F32, name="ps", tag="ps", bufs=8)
        for _ in range(NWARM_BIG):
            nc.tensor.matmul(ps_warm[:, 0:256], wrm[:, 0:128],
                             wrm[:, 128:384], start=True, stop=True)
        for _ in range(NWARM_SMALL):
            nc.tensor.matmul(ps_warm[:, 0:64], wrm[:, 0:128],
                             wrm[:, 128:192], start=True, stop=True)

        def mm(ps, j, b, i):
            nc.tensor.matmul(
                ps,
                kq[:, WPOS[j // 4], 2 * i:2 * i + 2,
                   (j % 4) * 128:(j % 4) * 128 + 128],
                kq[:, 4 + b, 2 * i:2 * i + 2, :],
                start=(i == 0), stop=(i == 1), perf_mode=DR)

        def psh():
            return ppool.tile([128, 512], F32, name="ps", tag="ps", bufs=8)

        def evac(obs, ps, on_dve):
            if on_dve:
                nc.vector.tensor_copy(obs, ps)
            else:
                nc.scalar.activation(obs, ps, ID)

        # ---- phase A/B0: ALL of j0-7 on the b0 half first (x b1 is the
        #      3rd load and only needed at +3.4us -- receipt-lag-proof),
        #      interleaved across j so accumulate chains don't stall.
        #      psum is 8 single-bank [128,512] tiles; j0-7 b0 occupy all
        #      8 banks (ps_warm's bank recycles under psB[3]).  b0 half-
        #      evacs alternate DVE (even j) / ACT (odd j) so neither
        #      engine bursts ----
        psA = [psh() for _ in range(4)]
        for i in range(2):
            for j in range(4):
                mm(psA[j], j, 0, i)
        psB = [psh() for _ in range(4)]
        for i in range(2):
            for j in range(4, 8):
                mm(psB[j - 4], j, 0, i)

        obt = [opool.tile([128, 2 * B], FP8, name="ob", tag="ob", bufs=8)
               for _ in range(4)]
        for j in range(8):
            o = obt[j // 2][:, (j % 2) * B:(j % 2 + 1) * B]
            ps = psA[j] if j < 4 else psB[j - 4]
            evac(o[:, 0:512], ps, on_dve=(j % 2 == 0))

        # ---- phase A2: j0-7 on b1; each b1 half evacs on the engine
        #      opposite its b0 half, pairs store as 256KB DMAs ----
        for j in range(8):
            ps1 = psh()
            mm(ps1, j, 1, 0)
            mm(ps1, j, 1, 1)
            o = obt[j // 2][:, (j % 2) * B:(j % 2 + 1) * B]
            evac(o[:, 512:B], ps1, on_dve=(j % 2 == 1))
            if j % 2 == 1:
                nc.sync.dma_start(
                    outT[:, j - 1:j + 1, :],
                    obt[j // 2].rearrange("p (a c) -> p a c", a=2))

        # ---- phase B: j8-15 in the order b0i0, b1i0, b0i1, b1i1
        #      (accumulate chains separated by one matmul; the b0 half
        #      closes on the 3rd matmul and evacuates while b1 finishes).
        #      Half-evacs go b0->DVE, b1->ACT; stores are paired 256KB
        #      DMAs, the last two chunks store per-chunk with the final
        #      chunk split across both HWDGE rings ----
        for jp in range(4, NC // 2):
            last = jp == NC // 2 - 1
            ob = opool.tile([128, 2 * B], FP8, name="ob", tag="ob", bufs=8)
            for h in range(2):
                j = jp * 2 + h
                obs = ob[:, h * B:(h + 1) * B]
                ps0, ps1 = psh(), psh()
                mm(ps0, j, 0, 0)
                mm(ps1, j, 1, 0)
                mm(ps0, j, 0, 1)
                final = last and h == 1
                if not final:
                    evac(obs[:, 0:512], ps0, on_dve=True)
                    mm(ps1, j, 1, 1)
                    evac(obs[:, 512:B], ps1, on_dve=False)
                    if last:
                        # j14: one 128KB store; the sync issue chain
                        # (~0.63us), not the evacs, bounds this chunk
                        nc.sync.dma_start(outT[:, j, :], obs)
                else:
                    # very last chunk: b0 half evacs on ACT and stores
                    # via the ACT HWDGE ring (qActDynamicHW) while the
                    # b1 half evacs on the otherwise-idle DVE and
                    # stores 64KB on the SP ring -- two parallel issue
                    # chains cut ~0.4us off the drain tail
                    evac(obs[:, 0:512], ps0, on_dve=False)
                    mm(ps1, j, 1, 1)
                    nc.scalar.dma_start(outT[:, j, 0:512],
                                        obs[:, 0:512])
                    evac(obs[:, 512:B], ps1, on_dve=True)
                    nc.sync.dma_start(outT[:, j, 512:B],
                                      obs[:, 512:B])
            if not last:
                nc.sync.dma_start(
                    outT[:, jp * 2:(jp + 1) * 2, :],
                    ob.rearrange("p (a c) -> p a c", a=2))


def _build():
    global _NC_CACHE
    if _NC_CACHE is not None:
        return _NC_CACHE
    nc = bacc.Bacc("TRN2", target_bir_lowering=False, debug=False,
                   enable_asserts=False, num_devices=1)
    g8 = nc.dram_tensor("g8", [128, 2, KC, 512], FP8,
                        kind="ExternalInput").ap()
    xb1_8 = nc.dram_tensor("xb1", [128, 1, KC, 512], FP8,
                           kind="ExternalInput").ap()
    wt8 = nc.dram_tensor("wt8", [128, 3, KC, 512], FP8,
                         kind="ExternalInput").ap()
    outT = nc.dram_tensor("outT", [128, NC, B], FP8,
                          kind="ExternalOutput").ap()
    # raw (non-tile) warmup operand: read-only garbage, no producer
    wrm = nc.alloc_sbuf_tensor("wrm0", [128, 384], BF16).ap()
    with tile.TileContext(nc) as tc:
        _body(nc, tc, g8, xb1_8, wt8, outT, wrm)
    nc.compile()
    _NC_CACHE = nc
    return nc


def _prepare_inputs(x, weight, split_square_len):
    assert x.shape == (B, D) and weight.shape == (D, N)
    assert int(split_square_len) == L

    x = np.ascontiguousarray(x, dtype=np.float32)
    weight = np.ascontiguousarray(weight, dtype=np.float32)

    # bias = colsum(weight^2)/L in f32, matching the reference
    bias = (np.einsum("dn,dn->n", weight, weight, dtype=np.float32)
            / np.float32(L)).astype(np.float32)

    # reference's global per-tensor scales (f32 arithmetic to match jax)
    max_x = np.float32(max(np.abs(x).max(), np.float32(1.0)))
    sx = np.maximum(max_x / QMAX, np.float32(1e-12))
    max_w = np.float32(max(np.abs(weight).max(), np.abs(bias).max()))
    sw = np.maximum(max_w / QMAX, np.float32(1e-12))

    # ones/bias rank-1 term: c[n] = L * round(1/sx) * round(bias[n]/sw)
    # * sx*sw --- exact replication of the reference's bias-rows term,
    # added on HOST after the fp8 store (values ~512 would swamp e4m3).
    k1 = np.float32(np.round(np.float32(1.0) / sx))
    kb = np.round(bias / sw).astype(np.float32)
    c_scaled = (np.float32(L) * k1) * kb * (sx * sw)

    # block-packed SBUF layouts: [p, blk, k, col] with 2KB+ lines
    xT = np.ascontiguousarray(x.T).astype(E4M3)          # [D, B]
    x8p = np.ascontiguousarray(
        xT.reshape(KC, 128, BT, 512).transpose(1, 2, 0, 3))
    w_q = weight.astype(E4M3)                            # [D, N]

    in_maps = []
    for c in range(NCORES):
        wc = w_q[:, c * NS:(c + 1) * NS]                 # [D, NS]
        w8p = wc.reshape(KC, 128, NB, 512).transpose(1, 2, 0, 3)
        # gate = [w nb0 | x b0] fused into one 4KB-line transfer
        g8 = np.ascontiguousarray(
            np.stack([w8p[:, 0], x8p[:, 0]], axis=1))    # [128,2,KC,512]
        xb1 = np.ascontiguousarray(x8p[:, 1:2])          # [128,1,KC,512]
        wt8 = np.ascontiguousarray(w8p[:, 1:4])          # [128,3,KC,512]
        in_maps.append({"g8": g8, "xb1": xb1, "wt8": wt8})
    return in_maps, c_scaled


def _run(in_maps, **kwargs):
    nc = _build()
    return bass_utils.run_bass_kernel_spmd(
        nc, in_maps, core_ids=list(range(NCORES)), **kwargs)


def _finalize(res, c_scaled):
    parts = []
    for c in range(NCORES):
        o = res.results[c]["outT"]                   # [128, NC, B] fp8
        parts.append(np.asarray(o).transpose(1, 0, 2).reshape(NS, B))
    out = np.concatenate(parts, axis=0).astype(np.float32)   # [N, B]
    out += c_scaled[:, None]
    return np.ascontiguousarray(out.T)               # [B, N] f32


def kernel(x, weight, split_square_len):
    in_maps, c_scaled = _prepare_inputs(x, weight, split_square_len)
    res = None
    for attempt in range(3):
        try:
            res = _run(in_maps)
            break
        except Exception:
            # transient NRT_EXEC_UNIT_UNRECOVERABLE device wedges have been
            # observed on this fabric; a plain re-execute does not always
            # clear them, but tearing down the PJRT client (the in-process
            # equivalent of a fresh interpreter) does
            if attempt == 2:
                raise
            try:
                import jax
                import jax.extend as _jex
                jax.clear_caches()
                _jex.backend.clear_backends()
            except Exception:
                pass
            time.sleep(2.0)
    return _finalize(res, c_scaled)

